# revision 23
# baseline (speedup 1.0000x reference)
"""Trainium2 Bass kernel for nn_DeconvBlock (dynamic-weight transposed conv).

Computes, per sample b:
    w_b   = weight + sum_j feature[b,j] * (t_j * m_j)            (weight synthesis)
    out_b = conv_transpose2d(x_b, w_b, stride=2, pad=1, K=4)     (grouped over batch)
    out   = prelu(out_b + bias, a)

Strategy (data-parallel over batch, 8 cores x 2 samples):
  - conv_transpose(stride 2, K=4, P=1) decomposes into 4 output phases
    (py,px) in {0,1}^2; each phase output pixel is a sum of 4 "taps"
    (ky,kx), each tap a 1x1 conv (matmul over CIN=256) of a +-1 shifted x.
  - Operands are fp16 (PE streams 16-bit moving operands at 1 col/cycle
    vs 2 for fp32; accumulation stays fp32 in PSUM; measured rel err
    ~4e-4). 512 matmuls/core of [128x128] @ [128x512] ~= 115us, which is
    the PE roofline for the 4.3 GMAC/core workload.
  - Per-sample weights are synthesized on-device: j=0 fused on VectorE
    (scalar_tensor_tensor), j=1..3 scaled on ScalarE and accumulated with
    2x-mode fp16 tensor_tensor adds, pipelined against the weight DMAs.
  - Epilogue: ScalarE adds bias (Identity activation w/ per-partition
    bias), VectorE computes prelu(t) = max(t, a*t) in one fused op while
    interleaving the 4 phases into contiguous output rows for clean DMA.
  - Startup DMAs are emitted in priority order on the sync queue (the
    per-engine dynamic HW queue is FIFO, so order == priority); startup
    is HBM-bandwidth-bound at ~6MB of prerequisites.
"""

import numpy as np

import concourse.bass as bass
import concourse.mybir as mybir
from concourse import bacc
from concourse import bass_utils
from concourse.tile import TileContext

B, CIN, COUT, H, W, K, S = 16, 256, 128, 64, 64, 4, 2
NCORES = 8
BPC = B // NCORES  # samples per core
P = 128
NCH = CIN // P     # ic chunks of 128
HP = H + 2         # padded x height/width (zero border of 1)
NROW = 8           # output-phase rows per block
NYB = H // NROW    # row blocks per sample

# phase py -> ((ky, sy), ...): contribution x[y'+sy] * w[ky]
_TAPS = {0: ((1, 0), (3, -1)), 1: ((2, 0), (0, 1))}

_COMPILED = None


def _build():
    f32 = mybir.dt.float32
    f32r = mybir.dt.float32r
    f16 = mybir.dt.float16
    Alu = mybir.AluOpType
    Act = mybir.ActivationFunctionType

    nc = bacc.Bacc(
        "TRN2", target_bir_lowering=False, debug=False, num_devices=NCORES
    )
    x_d = nc.dram_tensor(
        "x_sh", (BPC, NCH, P, HP, HP), f16, kind="ExternalInput"
    ).ap()
    w5_d = nc.dram_tensor("w5", (P, 5, NCH, K, K, COUT), f16, kind="ExternalInput").ap()
    feat_d = nc.dram_tensor("featb", (P, BPC, 4), f32, kind="ExternalInput").ap()
    bias_d = nc.dram_tensor("biasb", (P, 1), f32, kind="ExternalInput").ap()
    a_d = nc.dram_tensor("ab", (P, 1), f32, kind="ExternalInput").ap()
    out_d = nc.dram_tensor(
        "out_sh", (BPC, COUT, H * S, W * S), f32, kind="ExternalOutput"
    ).ap()

    with TileContext(nc) as tc:
        with (
            tc.tile_pool(name="const", bufs=1) as const_pool,
            tc.tile_pool(name="tmj_pool", bufs=1) as tm_pool,
            tc.tile_pool(name="wsyn_pool", bufs=1) as wsyn_pool,
            tc.tile_pool(name="x_pool", bufs=1) as x_pool,
            tc.tile_pool(name="t_pool", bufs=6) as t_pool,
            tc.tile_pool(name="sm_pool", bufs=3) as sm_pool,
            tc.tile_pool(name="row_pool", bufs=4) as row_pool,
            tc.tile_pool(name="psum", bufs=8, space="PSUM") as psum_pool,
        ):
            feat_t = const_pool.tile([P, BPC, 4], f32)
            nc.sync.dma_start(feat_t[:], feat_d[:])
            bias_t = const_pool.tile([P, 1], f32)
            a_t = const_pool.tile([P, 1], f32)
            # warm the ScalarE activation table (Identity) during startup DMAs
            scratch_t = const_pool.tile([P, 1], f32)
            nc.vector.memset(scratch_t[:], 0.0)
            nc.scalar.activation(scratch_t[:], scratch_t[:], Act.Identity, scale=1.0)

            # ---- per-sample weight synthesis on VectorE ----
            # w_syn[s][p, c, ky, kx, oc] = base + sum_j f[s,j] * TM_j
            wsyn = []
            xt = []
            for s in range(BPC):
                w_s = wsyn_pool.tile(
                    [P, NCH, K, K, COUT], f16, name=f"wsyn{s}", tag=f"wsyn{s}"
                )
                wsyn.append(w_s)
                x_s = x_pool.tile(
                    [P, NCH, HP, HP], f16, name=f"xpad{s}", tag=f"xpad{s}"
                )
                xt.append(x_s)
            # Startup DMAs scattered across engine queues so the transfers run
            # in parallel (each engine's dynamic HW queue is FIFO-serial).
            # Critical chain: tm(c0,j0) -> stt -> tm(c0,j1..3) scalings.
            tmt = {}
            for c in range(NCH):
                for j in range(4):
                    tmt[(c, j)] = tm_pool.tile(
                        [P, K, K, COUT], f16, name=f"tm{c}{j}", tag=f"tm{c}{j}"
                    )
            nc.sync.dma_start(wsyn[0][:, 0], w5_d[:, 4, 0])
            for j in range(4):
                nc.sync.dma_start(tmt[(0, j)][:], w5_d[:, j, 0])
            # x sample 0 rows 0:18 gate the first two row-blocks
            nc.sync.dma_start(xt[0][:, 0, 0:18], x_d[0, 0, :, 0:18])
            nc.sync.dma_start(xt[0][:, 1, 0:18], x_d[0, 1, :, 0:18])
            # chunk-1 weights for sample 0
            nc.sync.dma_start(wsyn[0][:, 1], w5_d[:, 4, 1])
            for j in range(4):
                nc.sync.dma_start(tmt[(1, j)][:], w5_d[:, j, 1])
            nc.sync.dma_start(bias_t[:], bias_d[:])
            nc.sync.dma_start(a_t[:], a_d[:])

            # synthesis: sample 0 fully first (it alone gates the first MMs).
            # j=0 fused on VectorE; j=1..3 scaled on ScalarE (sm = TM_j*f) and
            # accumulated on VectorE with 2x-mode fp16 tensor_tensor adds.
            def synth(s, c):
                nc.vector.scalar_tensor_tensor(
                    wsyn[s][:, c],
                    tmt[(c, 0)][:],
                    feat_t[:, s, 0:1],
                    wsyn[s][:, c],
                    op0=Alu.mult,
                    op1=Alu.add,
                )
                for j in range(1, 4):
                    sm = sm_pool.tile([P, K, K, COUT], f16, name="sm", tag="sm")
                    nc.scalar.activation(
                        sm[:],
                        tmt[(c, j)][:],
                        Act.Identity,
                        scale=feat_t[:, s, j : j + 1],
                    )
                    nc.vector.tensor_tensor(
                        wsyn[s][:, c], wsyn[s][:, c], sm[:], op=Alu.add
                    )

            for c in range(NCH):
                synth(0, c)
                if c == 0:
                    nc.sync.dma_start(xt[0][:, 0, 18:42], x_d[0, 0, :, 18:42])
                    nc.sync.dma_start(xt[0][:, 1, 18:42], x_d[0, 1, :, 18:42])
                    nc.sync.dma_start(xt[0][:, 0, 42:HP], x_d[0, 0, :, 42:HP])
                    nc.sync.dma_start(xt[0][:, 1, 42:HP], x_d[0, 1, :, 42:HP])
            nc.sync.dma_start(xt[1][:, 0], x_d[1, 0])
            nc.sync.dma_start(xt[1][:, 1], x_d[1, 1])
            nc.sync.dma_start(wsyn[1][:, 0], w5_d[:, 4, 0])
            nc.sync.dma_start(wsyn[1][:, 1], w5_d[:, 4, 1])
            for c in range(NCH):
                synth(1, c)

            # ---- main conv loop ----
            # Row-blocks of 8; the final sample's last block is split into
            # two 4-row halves so the tail's ACT->prelu->DMA chain after the
            # last matmul is half as long.
            blocks = [(NROW * i, NROW) for i in range(NYB)]
            last_blocks = blocks[:-1] + [
                (NROW * (NYB - 1), 4),
                (NROW * (NYB - 1) + 4, 2),
                (NROW * (NYB - 1) + 6, 2),
            ]
            PPS = ((0, 0), (0, 1), (1, 0), (1, 1))

            def mm_group(s, by0, nr, py, px, ps, chunks, nk):
                k = 0
                for c in chunks:
                    for ky, sy in _TAPS[py]:
                        for kx, sx in _TAPS[px]:
                            rhs = xt[s][
                                :, c, 1 + sy + by0 : 1 + sy + by0 + nr,
                                1 + sx : 1 + sx + W,
                            ]
                            nc.tensor.matmul(
                                ps[:],
                                wsyn[s][:, c, ky, kx, :],
                                rhs,
                                start=(k == 0),
                                stop=(k == nk - 1),
                            )
                            k += 1

            def out_dma(s, by0, nr, py, row_t):
                oy0 = 2 * by0 + py
                nc.sync.dma_start(
                    out_d[s, :, oy0 : oy0 + 2 * nr - 1 : 2, :], row_t[:, :, py]
                )

            # Sample 0, block 0: chunk-split groups. The c0-only matmuls gate
            # on just the 2.5MB chunk-0 weights, pulling the PE start earlier;
            # c1 accumulates into a second bank merged in the epilogue.
            row_t0 = row_pool.tile(
                [P, NROW, 2, W, 2], f32, name="row_t0", tag="row_t"
            )
            ta_list = []
            for py, px in PPS:
                ps = psum_pool.tile([P, NROW, W], f32, name="ps", tag="ps")
                mm_group(0, 0, NROW, py, px, ps, (0,), 4)
                ta = t_pool.tile([P, NROW, W], f32, name="ta", tag="ta", bufs=4)
                nc.scalar.activation(ta[:], ps[:], Act.Identity, scale=1.0)
                ta_list.append(ta)
            for idx, (py, px) in enumerate(PPS):
                ps = psum_pool.tile([P, NROW, W], f32, name="ps", tag="ps")
                mm_group(0, 0, NROW, py, px, ps, (1,), 4)
                tu = t_pool.tile([P, NROW, W], f32, name="tu", tag="tu", bufs=4)
                # tu = (c1_psum + bias) + c0_part
                nc.vector.scalar_tensor_tensor(
                    tu[:], ps[:], bias_t[:], ta_list[idx][:],
                    op0=Alu.add, op1=Alu.add,
                )
                nc.vector.scalar_tensor_tensor(
                    row_t0[:, :, py, :, px], tu[:], a_t[:], tu[:],
                    op0=Alu.mult, op1=Alu.max,
                )
                if px == 1:
                    out_dma(0, 0, NROW, py, row_t0)

            for s in range(BPC):
                blist = last_blocks if s == BPC - 1 else blocks
                if s == 0:
                    blist = blist[1:]  # block 0 done above
                for by0, nr in blist:
                    # row_t free layout (y', py, x', px) == out rows
                    # [2*nr, 2*W] for oy in [2*by0, 2*(by0+nr))
                    row_t = row_pool.tile(
                        [P, nr, 2, W, 2], f32, name="row_t", tag="row_t"
                    )
                    for py in (0, 1):
                        for px in (0, 1):
                            ps = psum_pool.tile(
                                [P, nr, W], f32, name="ps", tag="ps"
                            )
                            mm_group(s, by0, nr, py, px, ps, range(NCH), 8)
                            tt = t_pool.tile([P, nr, W], f32, name="tt", tag="tt")
                            nc.scalar.activation(
                                tt[:], ps[:], Act.Identity, bias=bias_t[:], scale=1.0
                            )
                            # prelu(t) = max(t, a*t), interleaved into row_t
                            nc.vector.scalar_tensor_tensor(
                                row_t[:, :, py, :, px],
                                tt[:],
                                a_t[:],
                                tt[:],
                                op0=Alu.mult,
                                op1=Alu.max,
                            )
                        if px == 1:
                            out_dma(s, by0, nr, py, row_t)


    nc.compile()
    return nc


def _get_compiled():
    global _COMPILED
    if _COMPILED is None:
        _COMPILED = _build()
    return _COMPILED


def _prep_in_maps(inputs):
    x = np.asarray(inputs["x"], dtype=np.float32)
    xp = np.zeros((B, NCH, P, HP, HP), dtype=np.float16)
    xp[:, :, :, 1 : HP - 1, 1 : HP - 1] = x.reshape(B, NCH, P, H, W)
    feat = np.asarray(inputs["feature"], dtype=np.float32)
    w = np.asarray(inputs["weight"], dtype=np.float32)
    tms = [
        np.asarray(inputs[f"t_{n}"], dtype=np.float32)[0]
        * np.asarray(inputs[f"m_{n}"], dtype=np.float32)[0]
        for n in ("bayer", "quad", "nano", "qxq")
    ]
    w5 = np.stack(tms + [w], axis=0)  # (5, CIN, COUT, K, K)
    w5 = w5.reshape(5, NCH, P, COUT, K, K).transpose(2, 0, 1, 4, 5, 3)
    w5 = np.ascontiguousarray(w5.astype(np.float16))  # (P, 5, NCH, K, K, COUT)
    biasb = np.ascontiguousarray(
        np.asarray(inputs["bias"], dtype=np.float32).reshape(P, 1)
    )
    ab = np.ascontiguousarray(
        np.broadcast_to(
            np.asarray(inputs["prelu_a"], dtype=np.float32).reshape(1, 1), (P, 1)
        )
    )
    in_maps = []
    for i in range(NCORES):
        sl = slice(i * BPC, (i + 1) * BPC)
        in_maps.append(
            {
                "x_sh": xp[sl],
                "w5": w5,
                "featb": np.ascontiguousarray(
                    np.broadcast_to(feat[sl][None], (P, BPC, 4))
                ),
                "biasb": biasb,
                "ab": ab,
            }
        )
    return in_maps


def kernel(**inputs):
    nc = _get_compiled()
    in_maps = _prep_in_maps(inputs)
    res = bass_utils.run_bass_kernel_spmd(nc, in_maps, core_ids=list(range(NCORES)))
    return np.concatenate(
        [res.results[i]["out_sh"] for i in range(NCORES)], axis=0
    )
